# revision 11
# baseline (speedup 1.0000x reference)
"""Trainium2 Bass kernel for BaseLIDIA weighted overlap-add (fold) network.

Math (derived from the reference):
  out[t,ch,y,x] = 0.5 * img[t,ch,y,x] / cnt[t,y,x] + mean(noisy[t,ch])
  img[ch,y,x]   = sum_{i,j in 0..4} deno[t, (y+4-i)*536 + (x+4-j), ch*25+i*5+j]
                                    * w[t, (y+4-i)*536 + (x+4-j)]
  cnt[y,x]      = sum_{i,j in 0..4} w[t, (y+4-i)*536 + (x+4-j)]
(`inds` is unused by the reference; the pre/post scaling collapses so that the
only use of `noisy` is its raw per-channel mean.)

Sharding: 8 cores = 2 frames x 4 row-bands of 133 output rows. Each core gets
patch rows [133b, 133b+137) (4-row halo) of its frame.

Per-core on-device algorithm (patch columns q on SBUF partitions, host layout
[q, d, r] with r padded to 138 so every engine AP is unit-stride innermost):
  - load deno band tile [q<=128, d=75, r=138] bf16, w tile [q, 5*138]
  - wd = deno * w  (w broadcast over d as the OUTER free dim -> DVE runs in
    2x_1P packed mode; a small d-slice goes to GpSimd to balance)
  - img[x, ch, y] = PSUM accumulation of 25 shifted matmuls (one per fold tap
    (i,j)): stationary 0/1 shift matrix handles x+4-j, the rhs AP offset
    (4-i) handles y+4-i.  No intermediate tiles, no strided reduces.
  - cnt via the same 25 matmuls on w with 2.0-valued shift matrices
    (folds the final *0.5 into 1/(2 cnt))
  - rcnt = reciprocal_approx_fast(2 cnt); o = img * rcnt (DVE, rcnt broadcast
    over ch); o += mean (GpSimd); DMA out as [x, (ch,y)] bf16 — the host
    transposes to [ch, y, x] during assembly.
"""

import ml_dtypes
import numpy as np

import concourse.bass as bass
import concourse.mybir as mybir
import concourse.tile as tile
from concourse import bacc
from concourse.bass_utils import run_bass_kernel_spmd

F32 = mybir.dt.float32
BF16 = mybir.dt.bfloat16
AX = mybir.AxisListType
ALU = mybir.AluOpType
ACTF = mybir.ActivationFunctionType

PS = 5
PH = PW = 536
H = W = 532
PD = 75
NBAND = 4
BAND_Y = 133          # output rows per band
BAND_R = 137          # patch rows per band (halo of PS-1)
RP = 138              # padded patch-row pitch (even -> keeps bf16 2x packing)
NPIX_CH = H * W       # 283024, per-channel pixel count
FD = 3 * BAND_Y       # 399 free elements of the img/out tiles

# x-blocks: (x0, nx, nq)  with q-range [x0, x0 + nq)
XBLKS = [(0, 124, 128), (124, 124, 128), (248, 124, 128), (372, 124, 128),
         (496, 36, 40)]


def _ap_p(base: bass.AP, npart: int, extra_off: int, dims):
    """Custom strided view of a tile: partition dim of `base` overridden to
    `npart`, free dims replaced."""
    part = [[base.ap[0][0], npart]]
    return bass.AP(base.tensor, base.offset + extra_off, part + [list(d) for d in dims])


def build_program(reps: int = 1, ablate: str = ""):
    """Build (and compile) the single-core Bass program. SPMD: all 8 cores run
    it on their own band slice. Returns the Bacc object."""
    nc = bacc.Bacc("TRN2", target_bir_lowering=False, debug=False,
                   enable_asserts=False, num_devices=8)

    deno_d = nc.dram_tensor("deno", [PW, PD, RP], BF16, kind="ExternalInput")
    wt_d = nc.dram_tensor("wt", [128, len(XBLKS) * RP], BF16,
                          kind="ExternalInput")
    noisy_d = nc.dram_tensor("noisy", [3, H, W], BF16, kind="ExternalInput")
    out_d = nc.dram_tensor("out", [W, FD], BF16, kind="ExternalOutput")

    with tile.TileContext(nc) as tc:
        with (
            tc.tile_pool(name="const", bufs=1) as const_p,
            tc.tile_pool(name="deno", bufs=3) as deno_p,
            tc.tile_pool(name="wq", bufs=2) as wq_p,
            tc.tile_pool(name="small", bufs=2) as small_p,
            tc.tile_pool(name="o1", bufs=2) as o1_p,
            tc.tile_pool(name="stage", bufs=3) as stage_p,
            tc.tile_pool(name="noisy", bufs=1) as noisy_p,
            tc.tile_pool(name="psI", bufs=2, space=bass.MemorySpace.PSUM) as psI,
            tc.tile_pool(name="psC", bufs=2, space=bass.MemorySpace.PSUM) as psC,
            tc.tile_pool(name="psW", bufs=1, space=bass.MemorySpace.PSUM) as psW,
        ):
            # ---- constants ----
            # shift identities: shifts[j][q, m] = v iff q == m + 4 - j
            def mkshift(tag, j, v):
                sh = const_p.tile([128, 124], BF16, tag=tag)
                nc.gpsimd.memset(sh[:], 0.0)
                nc.gpsimd.affine_select(
                    out=sh[:], in_=sh[:], compare_op=ALU.not_equal, fill=v,
                    base=j - 4, pattern=[[-1, 124]], channel_multiplier=1)
                return sh
            shifts = [mkshift(f"shift{j}", j, 1.0) for j in range(PS)]
            shifts2 = [mkshift(f"shift2{j}", j, 2.0) for j in range(PS)]

            ones76 = const_p.tile([76, 1], BF16, tag="ones76")
            nc.gpsimd.memset(ones76[:], 1.0)
            onesrow = const_p.tile([1, 128], F32, tag="onesrow")
            nc.gpsimd.memset(onesrow[:], 1.0 / NPIX_CH)

            # ---- per-channel means of raw noisy ----
            sums = const_p.tile([1, 3], F32, tag="sums")
            for ch in range(3):
                npix = noisy_p.tile([76, 3724], BF16, tag="noisy")
                nc.sync.dma_start(
                    out=npix[:],
                    in_=bass.AP(noisy_d, ch * NPIX_CH, [[3724, 76], [1, 3724]]))
                msum = psW.tile([1, 512], F32, tag="psw")
                nchunk = (3724 + 511) // 512
                for ci in range(nchunk):
                    c0 = ci * 512
                    n = min(512, 3724 - c0)
                    nc.tensor.matmul(
                        out=msum[0:1, 0:n],
                        lhsT=ones76[:],
                        rhs=npix[:, c0:c0 + n],
                        start=(ci == 0), stop=(ci == nchunk - 1))
                nc.vector.tensor_reduce(
                    out=sums[0:1, ch:ch + 1], in_=msum[0:1, 0:512],
                    axis=AX.X, op=ALU.add)
            mrep_ps = psW.tile([128, 3], F32, tag="psw")
            nc.tensor.matmul(out=mrep_ps[:], lhsT=onesrow[:],
                             rhs=sums[:], start=True, stop=True)
            mean_rep = const_p.tile([128, 3], F32, tag="mean_rep")
            nc.scalar.copy(mean_rep[:], mrep_ps[:])

            # ---- main loop ----
            # reps>1 wraps the body in a For_i hardware loop (for timing runs)
            # The For_i loop edge resets semaphores (no cross-iteration
            # pipelining), so each iteration pays the pipeline drain.  Unroll
            # UNROLL band-passes per iteration to amortize it.
            UNROLL = 4
            import contextlib
            loop_cm = (tc.For_i(0, (reps + UNROLL - 1) // UNROLL, 1,
                                staggered_reset=True)
                       if reps > 1 else contextlib.nullcontext())
            n_passes = UNROLL if reps > 1 else 1
            with loop_cm:
              for _pass in range(n_passes):
                wq = wq_p.tile([128, len(XBLKS) * RP], BF16, tag="wq")
                nc.sync.dma_start(out=wq[:], in_=wt_d[:, :])
                for b, (x0, nx, nq) in enumerate(XBLKS):
                    dt = deno_p.tile([128, PD * RP], BF16, tag="deno")
                    # load [q, d, r] slab in 3 d-chunks; the multiply streams
                    # chunk-by-chunk behind the DMA instead of waiting for the
                    # whole slab.
                    for (d0, d1) in ((0, 25), (25, 50), (50, PD)):
                        if "nodma" not in ablate:
                            nc.sync.dma_start(
                                out=dt[0:nq, d0 * RP:d1 * RP],
                                in_=bass.AP(deno_d, x0 * PD * RP + d0 * RP,
                                            [[PD * RP, nq], [1, (d1 - d0) * RP]]))
                        # wd = deno * w  (w broadcast over d: outer free dim,
                        # inner dim unit-stride -> DVE 2x packed mode)
                        if "nott" not in ablate:
                            nc.vector.tensor_tensor(
                                out=_ap_p(dt[:], nq, d0 * RP,
                                          [[RP, d1 - d0], [1, RP]]),
                                in0=_ap_p(dt[:], nq, d0 * RP,
                                          [[RP, d1 - d0], [1, RP]]),
                                in1=_ap_p(wq[:], nq, b * RP,
                                          [[0, d1 - d0], [1, RP]]),
                                op=ALU.mult)

                    # img[x, (ch,y)]: 25 shift-matmuls accumulating in PSUM.
                    # tap (i,j): rhs = wd[q, d=ch*25+i*5+j, r=y+4-i]
                    # cnt first: it depends only on wq, so the PE can run it
                    # while DMA/DVE fill this block's slab.
                    if "nomm" in ablate:
                        ablate = ablate + " nofin" if "nofin" not in ablate else ablate
                    else:
                        img = psI.tile([124, FD], F32, tag="img")
                        cnt = psC.tile([124, BAND_Y], F32, tag="cnt")
                        # cnt (scaled by 2): the 25 fold taps on w
                        for j in range(PS):
                            for i in range(PS):
                                nc.tensor.matmul(
                                    out=cnt[0:nx, :],
                                    lhsT=shifts2[j][0:nq, 0:nx],
                                    rhs=_ap_p(wq[:], nq, b * RP + (4 - i),
                                              [[1, BAND_Y]]),
                                    start=(j == 0 and i == 0),
                                    stop=(j == PS - 1 and i == PS - 1))
                        for j in range(PS):
                            for i in range(PS):
                                nc.tensor.matmul(
                                    out=img[0:nx, :],
                                    lhsT=shifts[j][0:nq, 0:nx],
                                    rhs=_ap_p(dt[:], nq,
                                              (i * PS + j) * RP + (4 - i),
                                              [[25 * RP, 3], [1, BAND_Y]]),
                                    start=(j == 0 and i == 0),
                                    stop=(j == PS - 1 and i == PS - 1))

                    # finals. Engine roles keep every PE-feeding queue free of
                    # PE-consuming ops: DVE runs the multiply (+cheap recip of
                    # the early-available cnt), ACT evacuates img PSUM, GpSimd
                    # does the remaining elementwise, ACT issues the out DMA.
                    st = stage_p.tile([124, FD], BF16, tag="st")
                    if "nofin" not in ablate:
                        rcnt = small_p.tile([124, BAND_Y], F32, tag="rcnt")
                        o1 = o1_p.tile([124, FD], BF16, tag="o1")
                        nc.vector.reciprocal_approx_fast(
                            rcnt[0:nx, :], cnt[0:nx, :])
                        nc.scalar.copy(o1[0:nx, :], img[0:nx, :])
                        nc.gpsimd.tensor_tensor(
                            out=st[0:nx, :],
                            in0=o1[0:nx, :],
                            in1=_ap_p(rcnt[:], nx, 0, [[0, 3], [1, BAND_Y]]),
                            op=ALU.mult)
                        nc.gpsimd.tensor_tensor(
                            out=st[0:nx, :],
                            in0=st[0:nx, :],
                            in1=_ap_p(mean_rep[:], nx, 0, [[1, 3], [0, BAND_Y]]),
                            op=ALU.add)
                    else:
                        nc.gpsimd.memset(st[:], 0.0)
                    if "noout" not in ablate:
                        nc.scalar.dma_start(
                            out=bass.AP(out_d, x0 * FD, [[FD, nx], [1, FD]]),
                            in_=st[0:nx, :])

    nc.compile()
    return nc


_CACHE = {}


def _get_program(reps: int = 1, ablate: str = ""):
    key = (reps, ablate)
    if key not in _CACHE:
        _CACHE[key] = build_program(reps, ablate)
    return _CACHE[key]


def make_in_maps(noisy, deno, patch_weights):
    in_maps = []
    bf = ml_dtypes.bfloat16
    for core in range(8):
        t, b = divmod(core, NBAND)
        dband = deno[t].reshape(PH, PW, PD)[133 * b:133 * b + BAND_R]
        dband = dband.transpose(1, 2, 0)          # [q=536, d=75, r=137]
        dpad = np.zeros((PW, PD, RP), dtype=bf)
        dpad[:, :, :BAND_R] = dband.astype(bf)
        wband = patch_weights[t, :, 0].reshape(PH, PW)[133 * b:133 * b + BAND_R]
        wband = wband.T                            # [q=536, r=137]
        wtile = np.zeros((128, len(XBLKS) * RP), dtype=bf)
        for blk, (x0, nx, nq) in enumerate(XBLKS):
            wtile[0:nq, blk * RP:blk * RP + BAND_R] = \
                wband[x0:x0 + nq].astype(bf)
        in_maps.append({
            "deno": dpad,
            "wt": wtile,
            "noisy": np.ascontiguousarray(noisy[t]).astype(bf),
        })
    return in_maps


def unpack_out(arr):
    """Device out [532, 399] bf16 -> [3, 133, 532] f32."""
    a = np.asarray(arr).astype(np.float32).reshape(W, 3, BAND_Y)
    return a.transpose(1, 2, 0)


def assemble(results):
    out = np.empty((2, 3, H, W), dtype=np.float32)
    for core in range(8):
        t, b = divmod(core, NBAND)
        out[t, :, 133 * b:133 * b + BAND_Y, :] = unpack_out(results[core]["out"])
    return out


def kernel(noisy, deno, patch_weights, inds=None, pixels_h=None, pixels_w=None,
           patches_h=None, patches_w=None, **_):
    noisy = np.asarray(noisy, dtype=np.float32)
    deno = np.asarray(deno, dtype=np.float32)
    patch_weights = np.asarray(patch_weights, dtype=np.float32)
    nc = _get_program()
    res = run_bass_kernel_spmd(nc, make_in_maps(noisy, deno, patch_weights),
                               core_ids=list(range(8)))
    return assemble(res.results)


# revision 12
# speedup vs baseline: 1.0750x; 1.0750x over previous
"""Trainium2 Bass kernel for BaseLIDIA weighted overlap-add (fold) network.

Math (derived from the reference):
  out[t,ch,y,x] = 0.5 * img[t,ch,y,x] / cnt[t,y,x] + mean(noisy[t,ch])
  img[ch,y,x]   = sum_{i,j in 0..4} deno[t, (y+4-i)*536 + (x+4-j), ch*25+i*5+j]
                                    * w[t, (y+4-i)*536 + (x+4-j)]
  cnt[y,x]      = sum_{i,j in 0..4} w[t, (y+4-i)*536 + (x+4-j)]
(`inds` is unused by the reference; the pre/post scaling collapses so that the
only use of `noisy` is its raw per-channel mean.)

Sharding: 8 cores = 2 frames x 4 row-bands of 133 output rows. Each core gets
patch rows [133b, 133b+137) (4-row halo) of its frame.

Per-core on-device algorithm (patch columns q on SBUF partitions, host layout
[q, d, r] with r padded to 138 so every engine AP is unit-stride innermost):
  - load deno band tile [q<=128, d=75, r=138] bf16, w tile [q, 5*138]
  - wd = deno * w  (w broadcast over d as the OUTER free dim -> DVE runs in
    2x_1P packed mode; a small d-slice goes to GpSimd to balance)
  - img[x, ch, y] = PSUM accumulation of 25 shifted matmuls (one per fold tap
    (i,j)): stationary 0/1 shift matrix handles x+4-j, the rhs AP offset
    (4-i) handles y+4-i.  No intermediate tiles, no strided reduces.
  - cnt via the same 25 matmuls on w with 2.0-valued shift matrices
    (folds the final *0.5 into 1/(2 cnt))
  - rcnt = reciprocal_approx_fast(2 cnt); o = img * rcnt (DVE, rcnt broadcast
    over ch); o += mean (GpSimd); DMA out as [x, (ch,y)] bf16 — the host
    transposes to [ch, y, x] during assembly.
"""

import ml_dtypes
import numpy as np

import concourse.bass as bass
import concourse.mybir as mybir
import concourse.tile as tile
from concourse import bacc
from concourse.bass_utils import run_bass_kernel_spmd

F32 = mybir.dt.float32
BF16 = mybir.dt.bfloat16
AX = mybir.AxisListType
ALU = mybir.AluOpType
ACTF = mybir.ActivationFunctionType

PS = 5
PH = PW = 536
H = W = 532
PD = 75
NBAND = 4
BAND_Y = 133          # output rows per band
BAND_R = 137          # patch rows per band (halo of PS-1)
RP = 138              # padded patch-row pitch (even -> keeps bf16 2x packing)
NPIX_CH = H * W       # 283024, per-channel pixel count
FD = 3 * BAND_Y       # 399 free elements of the img/out tiles

# x-blocks: (x0, nx, nq)  with q-range [x0, x0 + nq)
XBLKS = [(0, 124, 128), (124, 124, 128), (248, 124, 128), (372, 124, 128),
         (496, 36, 40)]


def _ap_p(base: bass.AP, npart: int, extra_off: int, dims):
    """Custom strided view of a tile: partition dim of `base` overridden to
    `npart`, free dims replaced."""
    part = [[base.ap[0][0], npart]]
    return bass.AP(base.tensor, base.offset + extra_off, part + [list(d) for d in dims])


def build_program(reps: int = 1, ablate: str = ""):
    """Build (and compile) the single-core Bass program. SPMD: all 8 cores run
    it on their own band slice. Returns the Bacc object."""
    nc = bacc.Bacc("TRN2", target_bir_lowering=False, debug=False,
                   enable_asserts=False, num_devices=8)

    deno_d = nc.dram_tensor("deno", [PW, PD, RP], BF16, kind="ExternalInput")
    wt_d = nc.dram_tensor("wt", [128, len(XBLKS) * RP], BF16,
                          kind="ExternalInput")
    noisy_d = nc.dram_tensor("noisy", [3, H, W], BF16, kind="ExternalInput")
    out_d = nc.dram_tensor("out", [W, FD], BF16, kind="ExternalOutput")

    with tile.TileContext(nc) as tc:
        with (
            tc.tile_pool(name="const", bufs=1) as const_p,
            tc.tile_pool(name="deno", bufs=3) as deno_p,
            tc.tile_pool(name="wq", bufs=2) as wq_p,
            tc.tile_pool(name="small", bufs=2) as small_p,
            tc.tile_pool(name="o1", bufs=2) as o1_p,
            tc.tile_pool(name="stage", bufs=3) as stage_p,
            tc.tile_pool(name="noisy", bufs=1) as noisy_p,
            tc.tile_pool(name="psI", bufs=2, space=bass.MemorySpace.PSUM) as psI,
            tc.tile_pool(name="psC", bufs=2, space=bass.MemorySpace.PSUM) as psC,
            tc.tile_pool(name="psW", bufs=1, space=bass.MemorySpace.PSUM) as psW,
        ):
            # ---- constants ----
            # shift identities: shifts[j][q, m] = v iff q == m + 4 - j
            def mkshift(tag, j, v):
                sh = const_p.tile([128, 124], BF16, tag=tag)
                nc.gpsimd.memset(sh[:], 0.0)
                nc.gpsimd.affine_select(
                    out=sh[:], in_=sh[:], compare_op=ALU.not_equal, fill=v,
                    base=j - 4, pattern=[[-1, 124]], channel_multiplier=1)
                return sh
            shifts = [mkshift(f"shift{j}", j, 1.0) for j in range(PS)]
            shifts2 = [mkshift(f"shift2{j}", j, 2.0) for j in range(PS)]

            ones76 = const_p.tile([76, 1], BF16, tag="ones76")
            nc.gpsimd.memset(ones76[:], 1.0)
            onesrow = const_p.tile([1, 128], F32, tag="onesrow")
            nc.gpsimd.memset(onesrow[:], 1.0 / NPIX_CH)

            # ---- per-channel means of raw noisy ----
            sums = const_p.tile([1, 3], F32, tag="sums")
            for ch in range(3):
                npix = noisy_p.tile([76, 3724], BF16, tag="noisy")
                nc.sync.dma_start(
                    out=npix[:],
                    in_=bass.AP(noisy_d, ch * NPIX_CH, [[3724, 76], [1, 3724]]))
                msum = psW.tile([1, 512], F32, tag="psw")
                nchunk = (3724 + 511) // 512
                for ci in range(nchunk):
                    c0 = ci * 512
                    n = min(512, 3724 - c0)
                    nc.tensor.matmul(
                        out=msum[0:1, 0:n],
                        lhsT=ones76[:],
                        rhs=npix[:, c0:c0 + n],
                        start=(ci == 0), stop=(ci == nchunk - 1))
                nc.vector.tensor_reduce(
                    out=sums[0:1, ch:ch + 1], in_=msum[0:1, 0:512],
                    axis=AX.X, op=ALU.add)
            mrep_ps = psW.tile([128, 3], F32, tag="psw")
            nc.tensor.matmul(out=mrep_ps[:], lhsT=onesrow[:],
                             rhs=sums[:], start=True, stop=True)
            mean_rep = const_p.tile([128, 3], F32, tag="mean_rep")
            nc.scalar.copy(mean_rep[:], mrep_ps[:])

            # ---- main loop ----
            # reps>1 wraps the body in a For_i hardware loop (for timing runs)
            # The For_i loop edge resets semaphores (no cross-iteration
            # pipelining), so each iteration pays the pipeline drain.  Unroll
            # UNROLL band-passes per iteration to amortize it.
            UNROLL = 4
            import contextlib
            loop_cm = (tc.For_i(0, (reps + UNROLL - 1) // UNROLL, 1)
                       if reps > 1 else contextlib.nullcontext())
            n_passes = UNROLL if reps > 1 else 1
            with loop_cm:
              for _pass in range(n_passes):
                wq = wq_p.tile([128, len(XBLKS) * RP], BF16, tag="wq")
                nc.sync.dma_start(out=wq[:], in_=wt_d[:, :])
                for b, (x0, nx, nq) in enumerate(XBLKS):
                    dt = deno_p.tile([128, PD * RP], BF16, tag="deno")
                    # load [q, d, r] slab in 3 d-chunks; the multiply streams
                    # chunk-by-chunk behind the DMA instead of waiting for the
                    # whole slab.
                    for (d0, d1) in ((0, 25), (25, 50), (50, PD)):
                        if "nodma" not in ablate:
                            nc.sync.dma_start(
                                out=dt[0:nq, d0 * RP:d1 * RP],
                                in_=bass.AP(deno_d, x0 * PD * RP + d0 * RP,
                                            [[PD * RP, nq], [1, (d1 - d0) * RP]]))
                        # wd = deno * w  (w broadcast over d: outer free dim,
                        # inner dim unit-stride -> DVE 2x packed mode)
                        if "nott" not in ablate:
                            nc.vector.tensor_tensor(
                                out=_ap_p(dt[:], nq, d0 * RP,
                                          [[RP, d1 - d0], [1, RP]]),
                                in0=_ap_p(dt[:], nq, d0 * RP,
                                          [[RP, d1 - d0], [1, RP]]),
                                in1=_ap_p(wq[:], nq, b * RP,
                                          [[0, d1 - d0], [1, RP]]),
                                op=ALU.mult)

                    # img[x, (ch,y)]: 25 shift-matmuls accumulating in PSUM.
                    # tap (i,j): rhs = wd[q, d=ch*25+i*5+j, r=y+4-i]
                    # cnt first: it depends only on wq, so the PE can run it
                    # while DMA/DVE fill this block's slab.
                    if "nomm" in ablate:
                        ablate = ablate + " nofin" if "nofin" not in ablate else ablate
                    else:
                        img = psI.tile([124, FD], F32, tag="img")
                        cnt = psC.tile([124, BAND_Y], F32, tag="cnt")
                        # cnt (scaled by 2): the 25 fold taps on w
                        for j in range(PS):
                            for i in range(PS):
                                nc.tensor.matmul(
                                    out=cnt[0:nx, :],
                                    lhsT=shifts2[j][0:nq, 0:nx],
                                    rhs=_ap_p(wq[:], nq, b * RP + (4 - i),
                                              [[1, BAND_Y]]),
                                    start=(j == 0 and i == 0),
                                    stop=(j == PS - 1 and i == PS - 1))
                        for j in range(PS):
                            for i in range(PS):
                                nc.tensor.matmul(
                                    out=img[0:nx, :],
                                    lhsT=shifts[j][0:nq, 0:nx],
                                    rhs=_ap_p(dt[:], nq,
                                              (i * PS + j) * RP + (4 - i),
                                              [[25 * RP, 3], [1, BAND_Y]]),
                                    start=(j == 0 and i == 0),
                                    stop=(j == PS - 1 and i == PS - 1))

                    # finals. Engine roles keep every PE-feeding queue free of
                    # PE-consuming ops: DVE runs the multiply (+cheap recip of
                    # the early-available cnt), ACT evacuates img PSUM, GpSimd
                    # does the remaining elementwise, ACT issues the out DMA.
                    st = stage_p.tile([124, FD], BF16, tag="st")
                    if "nofin" not in ablate:
                        rcnt = small_p.tile([124, BAND_Y], F32, tag="rcnt")
                        o1 = o1_p.tile([124, FD], BF16, tag="o1")
                        nc.vector.reciprocal_approx_fast(
                            rcnt[0:nx, :], cnt[0:nx, :])
                        nc.scalar.copy(o1[0:nx, :], img[0:nx, :])
                        nc.gpsimd.tensor_tensor(
                            out=st[0:nx, :],
                            in0=o1[0:nx, :],
                            in1=_ap_p(rcnt[:], nx, 0, [[0, 3], [1, BAND_Y]]),
                            op=ALU.mult)
                        nc.gpsimd.tensor_tensor(
                            out=st[0:nx, :],
                            in0=st[0:nx, :],
                            in1=_ap_p(mean_rep[:], nx, 0, [[1, 3], [0, BAND_Y]]),
                            op=ALU.add)
                    else:
                        nc.gpsimd.memset(st[:], 0.0)
                    if "noout" not in ablate:
                        nc.scalar.dma_start(
                            out=bass.AP(out_d, x0 * FD, [[FD, nx], [1, FD]]),
                            in_=st[0:nx, :])

    nc.compile()
    return nc


_CACHE = {}


def _get_program(reps: int = 1, ablate: str = ""):
    key = (reps, ablate)
    if key not in _CACHE:
        _CACHE[key] = build_program(reps, ablate)
    return _CACHE[key]


def make_in_maps(noisy, deno, patch_weights):
    in_maps = []
    bf = ml_dtypes.bfloat16
    for core in range(8):
        t, b = divmod(core, NBAND)
        dband = deno[t].reshape(PH, PW, PD)[133 * b:133 * b + BAND_R]
        dband = dband.transpose(1, 2, 0)          # [q=536, d=75, r=137]
        dpad = np.zeros((PW, PD, RP), dtype=bf)
        dpad[:, :, :BAND_R] = dband.astype(bf)
        wband = patch_weights[t, :, 0].reshape(PH, PW)[133 * b:133 * b + BAND_R]
        wband = wband.T                            # [q=536, r=137]
        wtile = np.zeros((128, len(XBLKS) * RP), dtype=bf)
        for blk, (x0, nx, nq) in enumerate(XBLKS):
            wtile[0:nq, blk * RP:blk * RP + BAND_R] = \
                wband[x0:x0 + nq].astype(bf)
        in_maps.append({
            "deno": dpad,
            "wt": wtile,
            "noisy": np.ascontiguousarray(noisy[t]).astype(bf),
        })
    return in_maps


def unpack_out(arr):
    """Device out [532, 399] bf16 -> [3, 133, 532] f32."""
    a = np.asarray(arr).astype(np.float32).reshape(W, 3, BAND_Y)
    return a.transpose(1, 2, 0)


def assemble(results):
    out = np.empty((2, 3, H, W), dtype=np.float32)
    for core in range(8):
        t, b = divmod(core, NBAND)
        out[t, :, 133 * b:133 * b + BAND_Y, :] = unpack_out(results[core]["out"])
    return out


def kernel(noisy, deno, patch_weights, inds=None, pixels_h=None, pixels_w=None,
           patches_h=None, patches_w=None, **_):
    noisy = np.asarray(noisy, dtype=np.float32)
    deno = np.asarray(deno, dtype=np.float32)
    patch_weights = np.asarray(patch_weights, dtype=np.float32)
    nc = _get_program()
    res = run_bass_kernel_spmd(nc, make_in_maps(noisy, deno, patch_weights),
                               core_ids=list(range(8)))
    return assemble(res.results)
